# revision 1
# baseline (speedup 1.0000x reference)
"""3D Haar DWT forward (J=2) on 8 Trainium2 NeuronCores.

Math: for even axis length N and 2-tap filters, the reference's
roll/pad/conv/fold collapses to a non-overlapping pairwise transform
    lo[j] = h0[0]*x[2j] + h0[1]*x[2j+1]
    hi[j] = h1[0]*x[2j] + h1[1]*x[2j+1]
applied along W, H, D. Channel order: c = 4*c_w + 2*c_h + c_d.
Output tuple: (ll, yh1, yh2) with shapes
    (4,1,32,64,64), (4,7,64,128,128), (4,7,32,64,64).

Sharding: pure data parallel, core = (n, dh) = batch x D-half. Each core
processes x[n, 0, dh*64:(dh+1)*64] -> (64, 256, 256); no halo because pairs
never cross shard boundaries.

Per-core pipeline (level 1), 16 blocks of 4 input planes:
  - DMA load A (128p, 4q x 2h x 256w): partition p holds plane rows p and
    128+p (h = row-half), so the H transform pairs adjacent partitions.
  - W stage on DVE: one scalar_tensor_tensor per lo/hi:
        W_s = even_w + (h_s[1]/h_s[0]) * odd_w
  - H stage on PE: fp32 matmul with a sparse 128x128 matrix pairing
    partitions (2j, 2j+1) -> 64 H-lo rows + 64 H-hi rows. Filter leads are
    folded into the matrix entries.
  - D stage on DVE: pairs adjacent planes (free dim): two STT ops
    (D-lo / D-hi) reading even planes from PSUM + odd planes from an SBUF
    copy made by ScalarE (DVE may read at most one PSUM operand).
  - ScalarE extracts the ll subband into a persistent SBUF collector.
  - Store the (128, 2048) result block verbatim; host reorders.
Level 2 runs the same pipeline on the SBUF-resident ll volume at 64
partitions. Host-side numpy decodes the block layouts into (ll, yh1, yh2).
"""

import time
import numpy as np

_CACHE = {}
N_CORES = 8


# ---------------------------------------------------------------------------
# runner (inlined; mirrors concourse.bass2jax.run_bass_via_pjrt but keeps the
# compiled callable for repeated executions with device-resident buffers)
# ---------------------------------------------------------------------------
class _SpmdRunner:
    def __init__(self, nc, n_cores):
        import jax
        import numpy as _np
        from jax.sharding import Mesh, PartitionSpec
        from jax.experimental.shard_map import shard_map
        import concourse.mybir as mybir
        from concourse.bass2jax import (
            _bass_exec_p,
            install_neuronx_cc_hook,
            partition_id_tensor,
        )

        install_neuronx_cc_hook()
        self.jax = jax
        self.nc = nc
        self.n_cores = n_cores
        self.partition_name = (
            nc.partition_id_tensor.name if nc.partition_id_tensor else None
        )

        in_names, out_names, out_avals = [], [], []
        for alloc in nc.m.functions[0].allocations:
            if not isinstance(alloc, mybir.MemoryLocationSet):
                continue
            name = alloc.memorylocations[0].name
            if alloc.kind == "ExternalInput":
                if name != self.partition_name:
                    in_names.append(name)
            elif alloc.kind == "ExternalOutput":
                out_names.append(name)
                out_avals.append(
                    jax.core.ShapedArray(
                        tuple(alloc.tensor_shape), mybir.dt.np(alloc.dtype)
                    )
                )
        self.in_names, self.out_names, self.out_avals = in_names, out_names, out_avals
        n_params = len(in_names)
        all_names = tuple(
            in_names + out_names + ([self.partition_name] if self.partition_name else [])
        )
        partition_name = self.partition_name

        def _body(*args):
            operands = list(args)
            if partition_name is not None:
                operands.append(partition_id_tensor())
            outs = _bass_exec_p.bind(
                *operands,
                out_avals=tuple(out_avals),
                in_names=all_names,
                out_names=tuple(out_names),
                lowering_input_output_aliases=(),
                sim_require_finite=False,
                sim_require_nnan=False,
                nc=nc,
            )
            return tuple(outs)

        devices = jax.devices()[:n_cores]
        assert len(devices) == n_cores, f"need {n_cores} cores, got {len(devices)}"
        self.mesh = Mesh(_np.asarray(devices), ("core",))
        in_specs = (PartitionSpec("core"),) * (n_params + len(out_names))
        out_specs = (PartitionSpec("core"),) * len(out_names)
        self.fn = jax.jit(
            shard_map(_body, mesh=self.mesh, in_specs=in_specs,
                      out_specs=out_specs, check_rep=False),
            keep_unused=True,
        )
        self.sharding = jax.sharding.NamedSharding(self.mesh, PartitionSpec("core"))

    def put_inputs(self, in_maps):
        concat = [
            np.concatenate(
                [np.ascontiguousarray(in_maps[c][n]) for c in range(self.n_cores)],
                axis=0,
            )
            for n in self.in_names
        ]
        for av in self.out_avals:
            concat.append(
                np.zeros((self.n_cores * av.shape[0], *av.shape[1:]), av.dtype)
            )
        dev = [self.jax.device_put(a, self.sharding) for a in concat]
        self.jax.block_until_ready(dev)
        return dev

    def run(self, dev_args):
        out = self.fn(*dev_args)
        self.jax.block_until_ready(out)
        return out

    def fetch(self, out_arrs):
        res = []
        for c in range(self.n_cores):
            m = {}
            for i, name in enumerate(self.out_names):
                a = np.asarray(out_arrs[i])
                m[name] = a.reshape(self.n_cores, *self.out_avals[i].shape)[c]
            res.append(m)
        return res


# ---------------------------------------------------------------------------
# kernel build
# ---------------------------------------------------------------------------
def _filter_scalars(h0, h1):
    g0, g1 = float(h0[0]), float(h0[1])
    k0, k1 = float(h1[0]), float(h1[1])
    assert abs(g0) > 1e-30 and abs(k0) > 1e-30, "lead filter taps must be nonzero"
    rlo = g1 / g0   # ratio for lo-filter STT stages (W and D)
    rhi = k1 / k0   # ratio for hi-filter STT stages
    gamma = k0 / g0  # post-scale for D-hi outputs (1.0 for Haar-like filters)
    return g0, g1, k0, k1, rlo, rhi, gamma


def _make_lhsT(h0, h1, half, lead):
    """Sparse pairing matrix, as matmul lhsT: out_j = sum_k lhsT[k, j] in_k.
    Rows j<half: lo filter; j>=half: hi filter; pairs (2j, 2j+1) -> j.
    All entries scaled by `lead`."""
    g0, g1, k0, k1, _, _, _ = _filter_scalars(h0, h1)
    m = np.zeros((2 * half, 2 * half), np.float32)
    j = np.arange(half)
    m[2 * j, j] = np.float32(g0 * lead)
    m[2 * j + 1, j] = np.float32(g1 * lead)
    m[2 * j, half + j] = np.float32(k0 * lead)
    m[2 * j + 1, half + j] = np.float32(k1 * lead)
    return m


def _build(h0, h1, repeat=1):
    import concourse.mybir as mybir
    from concourse.tile import TileContext
    from concourse import bacc

    g0, g1, k0, k1, rlo, rhi, gamma = _filter_scalars(h0, h1)
    ADD, MULT = mybir.AluOpType.add, mybir.AluOpType.mult
    f32 = mybir.dt.float32

    nc = bacc.Bacc("TRN2", target_bir_lowering=False, debug=False,
                   num_devices=N_CORES)

    x = nc.declare_dram_parameter("x", [64, 256, 256], f32, isOutput=False)
    mw = nc.declare_dram_parameter("mw", [2, 128, 128], f32, isOutput=False)
    mw2 = nc.declare_dram_parameter("mw2", [2, 64, 64], f32, isOutput=False)
    y1 = nc.declare_dram_parameter("y1", [16, 128, 2048], f32, isOutput=True)
    y2 = nc.declare_dram_parameter("y2", [4, 64, 2048], f32, isOutput=True)

    with TileContext(nc) as tc:
        with (
            tc.tile_pool(name="const", bufs=1) as constp,
            tc.tile_pool(name="io", bufs=3) as iop,
            tc.tile_pool(name="wst", bufs=2) as wp,
            tc.tile_pool(name="psum", bufs=2, space="PSUM") as psump,
            tc.tile_pool(name="qodd", bufs=2) as qop,
            tc.tile_pool(name="outp", bufs=3) as outp,
            tc.tile_pool(name="ll", bufs=1) as llp,
            tc.tile_pool(name="w2", bufs=1) as w2p,
        ):
            mw_sb = constp.tile([128, 256], f32, tag="mw")
            nc.sync.dma_start(out=mw_sb[:].rearrange("k (s m) -> k s m", s=2),
                              in_=mw[:].rearrange("s k m -> k s m"))
            mw2_sb = constp.tile([64, 128], f32, tag="mw2")
            nc.sync.dma_start(out=mw2_sb[:].rearrange("k (s m) -> k s m", s=2),
                              in_=mw2[:].rearrange("s k m -> k s m"))

            def body(_iv=None):
                LL = llp.tile([64, 8192], f32, tag="llc")  # (d1:32, h:2, w':128)
                for b in range(16):
                    # ---- load 4 planes: partition p = rows (p, 128+p) ----
                    A = iop.tile([128, 2048], f32, tag="a")
                    nc.sync.dma_start(
                        out=A[:].rearrange("p (q h w) -> p q h w", q=4, h=2),
                        in_=x[4 * b:4 * b + 4].rearrange(
                            "q (h p) w -> p q h w", h=2),
                    )
                    Av = A[:].rearrange("p (q h wp e) -> p q h wp e",
                                        q=4, h=2, e=2)
                    # ---- W stage ----
                    WL = wp.tile([128, 1024], f32, tag="wl")
                    WH = wp.tile([128, 1024], f32, tag="wh")
                    nc.vector.scalar_tensor_tensor(
                        out=WL[:].rearrange("p (q h wp) -> p q h wp", q=4, h=2),
                        in0=Av[:, :, :, :, 1], scalar=rlo,
                        in1=Av[:, :, :, :, 0], op0=MULT, op1=ADD)
                    nc.vector.scalar_tensor_tensor(
                        out=WH[:].rearrange("p (q h wp) -> p q h wp", q=4, h=2),
                        in0=Av[:, :, :, :, 1], scalar=rhi,
                        in1=Av[:, :, :, :, 0], op0=MULT, op1=ADD)
                    # ---- H stage: 4 matmuls into one PSUM block ----
                    Q = psump.tile([128, 2048], f32, tag="q")
                    for s, Wt in ((0, WL), (1, WH)):
                        for r in range(2):
                            nc.tensor.matmul(
                                Q[:, (s * 2 + r) * 512:(s * 2 + r + 1) * 512],
                                mw_sb[:, s * 128:(s + 1) * 128],
                                Wt[:, r * 512:(r + 1) * 512],
                                start=True, stop=True)
                    # Q layout: (s:2, r:2, e:2, hw:256); e = plane parity
                    Qv = Q[:].rearrange("p (s r e hw) -> p s r e hw",
                                        s=2, r=2, e=2)
                    QO = qop.tile([128, 1024], f32, tag="qo")
                    nc.scalar.copy(
                        out=QO[:].rearrange("p (s r hw) -> p s r hw", s=2, r=2),
                        in_=Qv[:, :, :, 1, :])
                    QOv = QO[:].rearrange("p (s r hw) -> p s r hw", s=2, r=2)
                    # ---- D stage ----
                    OUT = outp.tile([128, 2048], f32, tag="out")
                    OUTv = OUT[:].rearrange("p (ds s r hw) -> p ds s r hw",
                                            ds=2, s=2, r=2)
                    nc.vector.scalar_tensor_tensor(
                        out=OUTv[:, 0], in0=QOv, scalar=rlo,
                        in1=Qv[:, :, :, 0, :], op0=MULT, op1=ADD)
                    nc.vector.scalar_tensor_tensor(
                        out=OUTv[:, 1], in0=QOv, scalar=rhi,
                        in1=Qv[:, :, :, 0, :], op0=MULT, op1=ADD)
                    if gamma != 1.0:
                        nc.scalar.mul(OUT[:, 1024:2048], OUT[:, 1024:2048],
                                      gamma)
                    # ---- ll extract: OUT[0:64, (ds=0, s=0, r, hw)] ----
                    nc.scalar.copy(out=LL[:, b * 512:(b + 1) * 512],
                                   in_=OUT[0:64, 0:512])
                    nc.sync.dma_start(out=y1[b], in_=OUT[:])

                # ---------------- level 2 (64 partitions) ----------------
                LLv = LL[:].rearrange("p (dh wp e) -> p dh wp e", dh=64, e=2)
                W2L = w2p.tile([64, 4096], f32, tag="w2l")
                W2H = w2p.tile([64, 4096], f32, tag="w2h")
                nc.vector.scalar_tensor_tensor(
                    out=W2L[:].rearrange("p (dh wp) -> p dh wp", dh=64),
                    in0=LLv[:, :, :, 1], scalar=rlo,
                    in1=LLv[:, :, :, 0], op0=MULT, op1=ADD)
                nc.vector.scalar_tensor_tensor(
                    out=W2H[:].rearrange("p (dh wp) -> p dh wp", dh=64),
                    in0=LLv[:, :, :, 1], scalar=rhi,
                    in1=LLv[:, :, :, 0], op0=MULT, op1=ADD)
                for sp in range(4):
                    Q2 = psump.tile([64, 2048], f32, tag="q")
                    for s, W2t in ((0, W2L), (1, W2H)):
                        for m in range(2):
                            nc.tensor.matmul(
                                Q2[:, s * 1024 + m * 512:s * 1024 + (m + 1) * 512],
                                mw2_sb[:, s * 64:(s + 1) * 64],
                                W2t[:, sp * 1024 + m * 512:sp * 1024 + (m + 1) * 512],
                                start=True, stop=True)
                    # Q2 layout: (s:2, t:4, e:2, hw:128)
                    Q2v = Q2[:].rearrange("p (s t e hw) -> p s t e hw",
                                          s=2, t=4, e=2)
                    Q2O = qop.tile([64, 1024], f32, tag="qo2")
                    nc.scalar.copy(
                        out=Q2O[:].rearrange("p (s t hw) -> p s t hw", s=2, t=4),
                        in_=Q2v[:, :, :, 1, :])
                    Q2Ov = Q2O[:].rearrange("p (s t hw) -> p s t hw", s=2, t=4)
                    OUT2 = outp.tile([64, 2048], f32, tag="out2")
                    OUT2v = OUT2[:].rearrange("p (ds s t hw) -> p ds s t hw",
                                              ds=2, s=2, t=4)
                    nc.vector.scalar_tensor_tensor(
                        out=OUT2v[:, 0], in0=Q2Ov, scalar=rlo,
                        in1=Q2v[:, :, :, 0, :], op0=MULT, op1=ADD)
                    nc.vector.scalar_tensor_tensor(
                        out=OUT2v[:, 1], in0=Q2Ov, scalar=rhi,
                        in1=Q2v[:, :, :, 0, :], op0=MULT, op1=ADD)
                    if gamma != 1.0:
                        nc.scalar.mul(OUT2[:, 1024:2048], OUT2[:, 1024:2048],
                                      gamma)
                    nc.sync.dma_start(out=y2[sp], in_=OUT2[:])

            if repeat == 1:
                body()
            else:
                with tc.For_i(0, repeat, 1) as _i:
                    body(_i)

    nc.compile()
    return _SpmdRunner(nc, N_CORES)


def _get_runner(h0, h1, repeat=1):
    key = (tuple(np.asarray(h0, np.float64)), tuple(np.asarray(h1, np.float64)),
           repeat)
    if key not in _CACHE:
        _CACHE[key] = _build(h0, h1, repeat)
    return _CACHE[key]


# ---------------------------------------------------------------------------
# host-side shard / gather
# ---------------------------------------------------------------------------
def _make_in_maps(x, h0, h1):
    lhsT1 = _make_lhsT(h0, h1, 64, float(h0[0]) * float(h0[0]))
    lhsT1h = _make_lhsT(h0, h1, 64, float(h1[0]) * float(h0[0]))
    mw_np = np.stack([lhsT1, lhsT1h])  # (2, 128, 128); [s] scaled by leadW_s*leadD
    lhsT2 = _make_lhsT(h0, h1, 32, float(h0[0]) * float(h0[0]))
    lhsT2h = _make_lhsT(h0, h1, 32, float(h1[0]) * float(h0[0]))
    mw2_np = np.stack([lhsT2, lhsT2h])  # (2, 64, 64)
    in_maps = []
    for c in range(N_CORES):
        n, dh = c // 2, c % 2
        xs = np.ascontiguousarray(x[n, 0, dh * 64:(dh + 1) * 64])
        in_maps.append({"x": xs, "mw": mw_np, "mw2": mw2_np})
    return in_maps


def _decode_outputs(res):
    """res: per-core {y1: (16,128,2048), y2: (4,64,2048)} -> (ll, yh1, yh2)."""
    ll = np.empty((4, 1, 32, 64, 64), np.float32)
    yh1 = np.empty((4, 7, 64, 128, 128), np.float32)
    yh2 = np.empty((4, 7, 32, 64, 64), np.float32)
    for c in range(N_CORES):
        n, dh = c // 2, c % 2
        a = res[c]["y1"].reshape(16, 2, 64, 2, 2, 2, 2, 128)
        # dims: (b, ch, j, ds, s, r, h, w) -> (s, ch, ds, b, r, h, j, w)
        sub = np.transpose(a, (4, 1, 3, 0, 5, 6, 2, 7)).reshape(8, 32, 128, 128)
        yh1[n, :, dh * 32:(dh + 1) * 32] = sub[1:8]
        a2 = res[c]["y2"].reshape(4, 2, 32, 2, 2, 4, 2, 64)
        # dims: (sp, ch, j2, ds, s2, t, h, w) -> (s2, ch, ds, sp, t, h, j2, w)
        sub2 = np.transpose(a2, (4, 1, 3, 0, 5, 6, 2, 7)).reshape(8, 16, 64, 64)
        ll[n, 0, dh * 16:(dh + 1) * 16] = sub2[0]
        yh2[n, :, dh * 16:(dh + 1) * 16] = sub2[1:8]
    return ll, yh1, yh2


def kernel(x, h0, h1, J, **_unused):
    x = np.asarray(x, np.float32)
    h0 = np.asarray(h0, np.float32)
    h1 = np.asarray(h1, np.float32)
    assert int(J) == 2, f"kernel hardcodes J=2, got {J}"
    assert x.shape == (4, 1, 128, 256, 256), x.shape
    runner = _get_runner(h0, h1)
    dev = runner.put_inputs(_make_in_maps(x, h0, h1))
    out = runner.run(dev)
    return _decode_outputs(runner.fetch(out))


def _bench(x, h0, h1, repeats=(1, 9), iters=8):
    """Returns (per_iter_seconds, raw dict) using marginal cost between two
    in-kernel repeat counts to cancel the constant dispatch overhead."""
    x = np.asarray(x, np.float32)
    results = {}
    for rep in repeats:
        runner = _get_runner(h0, h1, rep)
        dev = runner.put_inputs(_make_in_maps(x, h0, h1))
        runner.run(dev)  # warmup (incl. jit compile)
        runner.run(dev)
        times = []
        for _ in range(iters):
            t0 = time.perf_counter()
            runner.run(dev)
            times.append(time.perf_counter() - t0)
        results[rep] = min(times)
    r0, r1 = repeats
    per_iter = (results[r1] - results[r0]) / (r1 - r0)
    return per_iter, results


# revision 16
# speedup vs baseline: 1.9290x; 1.9290x over previous
"""3D Haar DWT forward (J=2) on 8 Trainium2 NeuronCores.

Math: for even axis length N and 2-tap filters, the reference's
roll/pad/conv/fold collapses to a non-overlapping pairwise transform
    lo[j] = h0[0]*x[2j] + h0[1]*x[2j+1]
    hi[j] = h1[0]*x[2j] + h1[1]*x[2j+1]
applied along W, H, D. Channel order: c = 4*c_w + 2*c_h + c_d.
Output tuple: (ll, yh1, yh2) with shapes
    (4,1,32,64,64), (4,7,64,128,128), (4,7,32,64,64).

Sharding: pure data parallel, core = (n, dh) = batch x D-half. Each core
processes x[n, 0, dh*64:(dh+1)*64] -> (64, 256, 256); no halo because pairs
never cross shard boundaries.

Per-core pipeline (level 1), 16 blocks of 4 input planes:
  - DMA load A (128p, 4q x 2h x 256w): partition p holds plane rows p and
    128+p (h = row-half), so the H transform pairs adjacent partitions.
  - W stage on DVE: one scalar_tensor_tensor per lo/hi:
        W_s = even_w + (h_s[1]/h_s[0]) * odd_w
  - H stage on PE: fp32 matmul with a sparse 128x128 matrix pairing
    partitions (2j, 2j+1) -> 64 H-lo rows + 64 H-hi rows. Filter leads are
    folded into the matrix entries.
  - D stage on DVE: pairs adjacent planes (free dim): two STT ops
    (D-lo / D-hi) reading even planes from PSUM + odd planes from an SBUF
    copy made by ScalarE (DVE may read at most one PSUM operand).
  - ScalarE extracts the ll subband into a persistent SBUF collector.
  - Store the (128, 2048) result block verbatim; host reorders.
Level 2 runs the same pipeline on the SBUF-resident ll volume at 64
partitions. Host-side numpy decodes the block layouts into (ll, yh1, yh2).
"""

import time
import numpy as np

_CACHE = {}
N_CORES = 8


# ---------------------------------------------------------------------------
# runner (inlined; mirrors concourse.bass2jax.run_bass_via_pjrt but keeps the
# compiled callable for repeated executions with device-resident buffers)
# ---------------------------------------------------------------------------
class _SpmdRunner:
    def __init__(self, nc, n_cores):
        import jax
        import numpy as _np
        from jax.sharding import Mesh, PartitionSpec
        from jax.experimental.shard_map import shard_map
        import concourse.mybir as mybir
        from concourse.bass2jax import (
            _bass_exec_p,
            install_neuronx_cc_hook,
            partition_id_tensor,
        )

        install_neuronx_cc_hook()
        self.jax = jax
        self.nc = nc
        self.n_cores = n_cores
        self.partition_name = (
            nc.partition_id_tensor.name if nc.partition_id_tensor else None
        )

        in_names, out_names, out_avals = [], [], []
        for alloc in nc.m.functions[0].allocations:
            if not isinstance(alloc, mybir.MemoryLocationSet):
                continue
            name = alloc.memorylocations[0].name
            if alloc.kind == "ExternalInput":
                if name != self.partition_name:
                    in_names.append(name)
            elif alloc.kind == "ExternalOutput":
                out_names.append(name)
                out_avals.append(
                    jax.core.ShapedArray(
                        tuple(alloc.tensor_shape), mybir.dt.np(alloc.dtype)
                    )
                )
        self.in_names, self.out_names, self.out_avals = in_names, out_names, out_avals
        n_params = len(in_names)
        all_names = tuple(
            in_names + out_names + ([self.partition_name] if self.partition_name else [])
        )
        partition_name = self.partition_name

        def _body(*args):
            operands = list(args)
            if partition_name is not None:
                operands.append(partition_id_tensor())
            outs = _bass_exec_p.bind(
                *operands,
                out_avals=tuple(out_avals),
                in_names=all_names,
                out_names=tuple(out_names),
                lowering_input_output_aliases=(),
                sim_require_finite=False,
                sim_require_nnan=False,
                nc=nc,
            )
            return tuple(outs)

        devices = jax.devices()[:n_cores]
        assert len(devices) == n_cores, f"need {n_cores} cores, got {len(devices)}"
        self.mesh = Mesh(_np.asarray(devices), ("core",))
        in_specs = (PartitionSpec("core"),) * (n_params + len(out_names))
        out_specs = (PartitionSpec("core"),) * len(out_names)
        self.fn = jax.jit(
            shard_map(_body, mesh=self.mesh, in_specs=in_specs,
                      out_specs=out_specs, check_rep=False),
            keep_unused=True,
        )
        self.sharding = jax.sharding.NamedSharding(self.mesh, PartitionSpec("core"))

    def put_inputs(self, in_maps):
        concat = [
            np.concatenate(
                [np.ascontiguousarray(in_maps[c][n]) for c in range(self.n_cores)],
                axis=0,
            )
            for n in self.in_names
        ]
        for av in self.out_avals:
            concat.append(
                np.zeros((self.n_cores * av.shape[0], *av.shape[1:]), av.dtype)
            )
        dev = [self.jax.device_put(a, self.sharding) for a in concat]
        self.jax.block_until_ready(dev)
        return dev

    def run(self, dev_args):
        out = self.fn(*dev_args)
        self.jax.block_until_ready(out)
        return out

    def fetch(self, out_arrs):
        res = []
        for c in range(self.n_cores):
            m = {}
            for i, name in enumerate(self.out_names):
                a = np.asarray(out_arrs[i])
                m[name] = a.reshape(self.n_cores, *self.out_avals[i].shape)[c]
            res.append(m)
        return res


# ---------------------------------------------------------------------------
# kernel build
# ---------------------------------------------------------------------------
def _filter_scalars(h0, h1):
    g0, g1 = float(h0[0]), float(h0[1])
    k0, k1 = float(h1[0]), float(h1[1])
    assert abs(g0) > 1e-30 and abs(k0) > 1e-30, "lead filter taps must be nonzero"
    rlo = g1 / g0   # ratio for lo-filter STT stages (W and D)
    rhi = k1 / k0   # ratio for hi-filter STT stages
    gamma = k0 / g0  # post-scale for D-hi outputs (1.0 for Haar-like filters)
    return g0, g1, k0, k1, rlo, rhi, gamma


def _make_lhsT(h0, h1, half, lead):
    """Sparse pairing matrix, as matmul lhsT: out_j = sum_k lhsT[k, j] in_k.
    Rows j<half: lo filter; j>=half: hi filter; pairs (2j, 2j+1) -> j.
    All entries scaled by `lead`."""
    g0, g1, k0, k1, _, _, _ = _filter_scalars(h0, h1)
    m = np.zeros((2 * half, 2 * half), np.float32)
    j = np.arange(half)
    m[2 * j, j] = np.float32(g0 * lead)
    m[2 * j + 1, j] = np.float32(g1 * lead)
    m[2 * j, half + j] = np.float32(k0 * lead)
    m[2 * j + 1, half + j] = np.float32(k1 * lead)
    return m


def _build(h0, h1, repeat=1):
    import concourse.mybir as mybir
    from concourse.tile import TileContext
    from concourse import bacc

    g0, g1, k0, k1, rlo, rhi, gamma = _filter_scalars(h0, h1)
    ADD, MULT = mybir.AluOpType.add, mybir.AluOpType.mult
    f32 = mybir.dt.float32

    nc = bacc.Bacc("TRN2", target_bir_lowering=False, debug=False,
                   num_devices=N_CORES)

    x = nc.declare_dram_parameter("x", [64, 256, 256], f32, isOutput=False)
    mw = nc.declare_dram_parameter("mw", [2, 128, 128], f32, isOutput=False)
    mw2 = nc.declare_dram_parameter("mw2", [2, 64, 64], f32, isOutput=False)
    y1 = nc.declare_dram_parameter("y1", [16, 128, 2048], f32, isOutput=True)
    y2 = nc.declare_dram_parameter("y2", [4, 64, 2048], f32, isOutput=True)

    with TileContext(nc) as tc:
        with (
            tc.tile_pool(name="const", bufs=1) as constp,
            tc.tile_pool(name="io", bufs=4) as iop,
            tc.tile_pool(name="wst", bufs=3) as wp,
            tc.tile_pool(name="psum", bufs=2, space="PSUM") as psump,
            tc.tile_pool(name="qodd", bufs=3) as qop,
            tc.tile_pool(name="outp", bufs=4) as outp,
            tc.tile_pool(name="ll", bufs=2) as llp,
            tc.tile_pool(name="w2", bufs=2) as w2p,
        ):
            mw_sb = constp.tile([128, 256], f32, tag="mw")
            nc.sync.dma_start(out=mw_sb[:].rearrange("k (s m) -> k s m", s=2),
                              in_=mw[:].rearrange("s k m -> k s m"))
            mw2_sb = constp.tile([64, 128], f32, tag="mw2")
            nc.sync.dma_start(out=mw2_sb[:].rearrange("k (s m) -> k s m", s=2),
                              in_=mw2[:].rearrange("s k m -> k s m"))

            def level2_subphase(sp, LL):
                """Level-2 on ll planes d1 in [8sp, 8sp+8); LL: (64, 2048) =
                (d1:8, h:2, w':128). Runs interleaved after every 4 L1 blocks."""
                LLv = LL[:].rearrange("p (dh wp e) -> p dh wp e", dh=16, e=2)
                W2L = w2p.tile([64, 1024], f32, tag="w2l")
                W2H = w2p.tile([64, 1024], f32, tag="w2h")
                nc.vector.scalar_tensor_tensor(
                    out=W2L[:].rearrange("p (dh wp) -> p dh wp", dh=16),
                    in0=LLv[:, :, :, 1], scalar=rlo,
                    in1=LLv[:, :, :, 0], op0=MULT, op1=ADD)
                nc.vector.scalar_tensor_tensor(
                    out=W2H[:].rearrange("p (dh wp) -> p dh wp", dh=16),
                    in0=LLv[:, :, :, 1], scalar=rhi,
                    in1=LLv[:, :, :, 0], op0=MULT, op1=ADD)
                Q2 = psump.tile([64, 2048], f32, tag="q")
                for s, W2t in ((0, W2L), (1, W2H)):
                    for m in range(2):
                        nc.tensor.matmul(
                            Q2[:, s * 1024 + m * 512:s * 1024 + (m + 1) * 512],
                            mw2_sb[:, s * 64:(s + 1) * 64],
                            W2t[:, m * 512:(m + 1) * 512],
                            start=True, stop=True)
                # Q2 layout: (s:2, t:4, e:2, hw:128)
                Q2v = Q2[:].rearrange("p (s t e hw) -> p s t e hw", s=2, t=4, e=2)
                Q2O = qop.tile([64, 1024], f32, tag="qo2")
                Q2E = qop.tile([64, 1024], f32, tag="qe2")
                nc.scalar.copy(
                    out=Q2O[:].rearrange("p (s t hw) -> p s t hw", s=2, t=4),
                    in_=Q2v[:, :, :, 1, :])
                nc.scalar.copy(
                    out=Q2E[:].rearrange("p (s t hw) -> p s t hw", s=2, t=4),
                    in_=Q2v[:, :, :, 0, :])
                OUT2 = outp.tile([64, 2048], f32, tag="out2")
                nc.vector.scalar_tensor_tensor(
                    out=OUT2[:, 0:1024], in0=Q2O[:], scalar=rlo,
                    in1=Q2E[:], op0=MULT, op1=ADD)
                nc.vector.scalar_tensor_tensor(
                    out=OUT2[:, 1024:2048], in0=Q2O[:], scalar=rhi,
                    in1=Q2E[:], op0=MULT, op1=ADD)
                if gamma != 1.0:
                    nc.scalar.mul(OUT2[:, 1024:2048], OUT2[:, 1024:2048], gamma)
                nc.gpsimd.dma_start(out=y2[sp], in_=OUT2[:])

            def body(_iv=None):
                LL = None
                ll_groups = {}
                for b in range(16):
                    if b % 4 == 0:
                        LL = llp.tile([64, 2048], f32, tag="llc")
                        ll_groups[b // 4] = LL
                    # ---- load 4 planes: partition p = rows (p, 128+p) ----
                    A = iop.tile([128, 2048], f32, tag="a")
                    nc.sync.dma_start(
                        out=A[:].rearrange("p (q h w) -> p q h w", q=4, h=2),
                        in_=x[4 * b:4 * b + 4].rearrange(
                            "q (h p) w -> p q h w", h=2),
                    )
                    Av = A[:].rearrange("p (q h wp e) -> p q h wp e",
                                        q=4, h=2, e=2)
                    # ---- W stage ----
                    WL = wp.tile([128, 1024], f32, tag="wl")
                    WH = wp.tile([128, 1024], f32, tag="wh")
                    nc.vector.scalar_tensor_tensor(
                        out=WL[:].rearrange("p (q h wp) -> p q h wp", q=4, h=2),
                        in0=Av[:, :, :, :, 1], scalar=rlo,
                        in1=Av[:, :, :, :, 0], op0=MULT, op1=ADD)
                    nc.vector.scalar_tensor_tensor(
                        out=WH[:].rearrange("p (q h wp) -> p q h wp", q=4, h=2),
                        in0=Av[:, :, :, :, 1], scalar=rhi,
                        in1=Av[:, :, :, :, 0], op0=MULT, op1=ADD)
                    # ---- H stage: 4 matmuls into one PSUM block ----
                    Q = psump.tile([128, 2048], f32, tag="q")
                    for s, Wt in ((0, WL), (1, WH)):
                        for r in range(2):
                            nc.tensor.matmul(
                                Q[:, (s * 2 + r) * 512:(s * 2 + r + 1) * 512],
                                mw_sb[:, s * 128:(s + 1) * 128],
                                Wt[:, r * 512:(r + 1) * 512],
                                start=True, stop=True)
                    # Q layout: (s:2, r:2, e:2, hw:256); e = plane parity.
                    # ScalarE de-interleaves both parities out of PSUM so the
                    # DVE D-stage ops see only flat contiguous SBUF operands.
                    Qv = Q[:].rearrange("p (s r e hw) -> p s r e hw",
                                        s=2, r=2, e=2)
                    QO = qop.tile([128, 1024], f32, tag="qo")
                    QE = qop.tile([128, 1024], f32, tag="qe")
                    nc.scalar.copy(
                        out=QO[:].rearrange("p (s r hw) -> p s r hw", s=2, r=2),
                        in_=Qv[:, :, :, 1, :])
                    nc.scalar.copy(
                        out=QE[:].rearrange("p (s r hw) -> p s r hw", s=2, r=2),
                        in_=Qv[:, :, :, 0, :])
                    # ---- D stage ----
                    OUT = outp.tile([128, 2048], f32, tag="out")
                    nc.vector.scalar_tensor_tensor(
                        out=OUT[:, 0:1024], in0=QO[:], scalar=rlo,
                        in1=QE[:], op0=MULT, op1=ADD)
                    nc.vector.scalar_tensor_tensor(
                        out=OUT[:, 1024:2048], in0=QO[:], scalar=rhi,
                        in1=QE[:], op0=MULT, op1=ADD)
                    if gamma != 1.0:
                        nc.scalar.mul(OUT[:, 1024:2048], OUT[:, 1024:2048],
                                      gamma)
                    # ---- ll extract: OUT[0:64, (ds=0, s=0, r, hw)] ----
                    nc.scalar.copy(out=LL[:, (b % 4) * 512:(b % 4 + 1) * 512],
                                   in_=OUT[0:64, 0:512])
                    nc.gpsimd.dma_start(out=y1[b], in_=OUT[:])
                    # run L2 sub-phase sp once its ll data is 4 blocks old, so
                    # engines never wait on a fresh dependency chain
                    if b % 4 == 3 and b >= 7:
                        level2_subphase(b // 4 - 1, ll_groups.pop(b // 4 - 1))
                level2_subphase(3, ll_groups.pop(3))

            if repeat == 1:
                body()
            else:
                with tc.For_i(0, repeat, 1) as _i:
                    body(_i)

    nc.compile()
    return _SpmdRunner(nc, N_CORES)


def _get_runner(h0, h1, repeat=1):
    key = (tuple(np.asarray(h0, np.float64)), tuple(np.asarray(h1, np.float64)),
           repeat)
    if key not in _CACHE:
        _CACHE[key] = _build(h0, h1, repeat)
    return _CACHE[key]


# ---------------------------------------------------------------------------
# host-side shard / gather
# ---------------------------------------------------------------------------
def _make_in_maps(x, h0, h1):
    lhsT1 = _make_lhsT(h0, h1, 64, float(h0[0]) * float(h0[0]))
    lhsT1h = _make_lhsT(h0, h1, 64, float(h1[0]) * float(h0[0]))
    mw_np = np.stack([lhsT1, lhsT1h])  # (2, 128, 128); [s] scaled by leadW_s*leadD
    lhsT2 = _make_lhsT(h0, h1, 32, float(h0[0]) * float(h0[0]))
    lhsT2h = _make_lhsT(h0, h1, 32, float(h1[0]) * float(h0[0]))
    mw2_np = np.stack([lhsT2, lhsT2h])  # (2, 64, 64)
    in_maps = []
    for c in range(N_CORES):
        n, dh = c // 2, c % 2
        xs = np.ascontiguousarray(x[n, 0, dh * 64:(dh + 1) * 64])
        in_maps.append({"x": xs, "mw": mw_np, "mw2": mw2_np})
    return in_maps


def _decode_outputs(res):
    """res: per-core {y1: (16,128,2048), y2: (4,64,2048)} -> (ll, yh1, yh2)."""
    ll = np.empty((4, 1, 32, 64, 64), np.float32)
    yh1 = np.empty((4, 7, 64, 128, 128), np.float32)
    yh2 = np.empty((4, 7, 32, 64, 64), np.float32)
    for c in range(N_CORES):
        n, dh = c // 2, c % 2
        a = res[c]["y1"].reshape(16, 2, 64, 2, 2, 2, 2, 128)
        # dims: (b, ch, j, ds, s, r, h, w) -> (s, ch, ds, b, r, h, j, w)
        sub = np.transpose(a, (4, 1, 3, 0, 5, 6, 2, 7)).reshape(8, 32, 128, 128)
        yh1[n, :, dh * 32:(dh + 1) * 32] = sub[1:8]
        a2 = res[c]["y2"].reshape(4, 2, 32, 2, 2, 4, 2, 64)
        # dims: (sp, ch, j2, ds, s2, t, h, w) -> (s2, ch, ds, sp, t, h, j2, w)
        sub2 = np.transpose(a2, (4, 1, 3, 0, 5, 6, 2, 7)).reshape(8, 16, 64, 64)
        ll[n, 0, dh * 16:(dh + 1) * 16] = sub2[0]
        yh2[n, :, dh * 16:(dh + 1) * 16] = sub2[1:8]
    return ll, yh1, yh2


def kernel(x, h0, h1, J, **_unused):
    x = np.asarray(x, np.float32)
    h0 = np.asarray(h0, np.float32)
    h1 = np.asarray(h1, np.float32)
    assert int(J) == 2, f"kernel hardcodes J=2, got {J}"
    assert x.shape == (4, 1, 128, 256, 256), x.shape
    runner = _get_runner(h0, h1)
    dev = runner.put_inputs(_make_in_maps(x, h0, h1))
    out = runner.run(dev)
    return _decode_outputs(runner.fetch(out))


def _bench(x, h0, h1, repeats=(1, 129), iters=8):
    """Returns (per_iter_seconds, raw dict) using marginal cost between two
    in-kernel repeat counts to cancel the constant dispatch overhead."""
    x = np.asarray(x, np.float32)
    results = {}
    for rep in repeats:
        runner = _get_runner(h0, h1, rep)
        dev = runner.put_inputs(_make_in_maps(x, h0, h1))
        runner.run(dev)  # warmup (incl. jit compile)
        runner.run(dev)
        times = []
        for _ in range(iters):
            t0 = time.perf_counter()
            runner.run(dev)
            times.append(time.perf_counter() - t0)
        results[rep] = min(times)
    r0, r1 = repeats
    per_iter = (results[r1] - results[r0]) / (r1 - r0)
    return per_iter, results
